# revision 8
# baseline (speedup 1.0000x reference)
"""Squared-Euclidean-distance kernel for Trainium2 (8 NeuronCores, SPMD).

Computes out[b,n,u] = sum_d (x[b,n,d] - w[d,u])^2 for
x [8, 4096, 128] f32, w [128, 1024] f32 -> out [8, 4096, 1024] f32,
via the algebraic identity |x|^2 + |w|^2 - 2 x.w.

Distribution: data-parallel over the batch dim — core c handles x[c]
([4096, 128] rows), w replicated. No cross-core communication.

Per-core device kernel:
  - host precomputes xt = x[c].T (d on partitions), wneg2 = -2w,
    x2 (per-point squared norms, laid out [128, 32] column-per-n-tile)
    and w2p (|w_u|^2 broadcast to all 128 partitions).
  - 32 n-tiles of 128 points: PSUM[128,1024] = xt_tile.T @ wneg2 (2
    matmuls of free-dim 512), then ScalarE adds x2 (per-partition bias)
    while copying PSUM->SBUF, VectorE adds w2p, DMA to HBM.
"""

import sys
import types

try:
    import concourse.bass as bass  # noqa: F401
except ImportError:  # fresh interpreter without the repo on sys.path
    sys.path.insert(0, "/opt/trn_rl_repo")

import numpy as np
import ml_dtypes

import concourse.bass as bass
import concourse.bacc as bacc
import concourse.tile as tile
import concourse.mybir as mybir
import concourse.bass_utils as bass_utils
from concourse.bass_utils import run_bass_kernel_spmd

B, N, D, U = 8, 4096, 128, 1024
N_CORES = 8
P = 128
N_TILES = N // P          # 32 n-tiles per core
U_HALF = 512              # PSUM bank = 512 f32
XT_CHUNK = 512            # xt loaded as 8 chunks of [128, 512]

# GEMM operand dtype: float32 (exact, 4 cyc/col), float32r (fp32 bits,
# full-rate 1 cyc/col, ~1e-4 rel accuracy), float16 (full rate, 2-byte
# inputs, ~3e-4) or bfloat16 (~2e-3).
# The |x|^2 / |w|^2 rank-1 terms always stay f32 (added outside the PE).
GEMM_DT = mybir.dt.float16
GEMM_NP = np.float16


def _np_of(dt):
    return {mybir.dt.float32: np.float32,
            mybir.dt.float32r: np.float32,
            mybir.dt.bfloat16: ml_dtypes.bfloat16}[dt]


def _install_ntff_hook():
    """Wire the NTFF profile hook the agent image leaves unconnected."""
    if "antenv.axon_hooks" in sys.modules:
        return
    try:
        from trn_agent_boot.trn_boot import _ntff_profile_via_ctypes
        hook = _ntff_profile_via_ctypes("/opt/axon/libaxon_pjrt.so")
    except Exception:
        hook = None
    mod = types.ModuleType("antenv.axon_hooks")
    mod.get_axon_ntff_profile_hook = lambda: hook
    mod.set_axon_ntff_profile_hook = lambda h: None
    sys.modules["antenv.axon_hooks"] = mod
    bass_utils.upload_artifacts = lambda tmpdir: f"local://{tmpdir}"


def build_bass(gemm_dt=None):
    """Build + compile the per-core Bass program (SPMD, same on all cores)."""
    gemm_dt = gemm_dt or GEMM_DT
    nc = bacc.Bacc("TRN2", target_bir_lowering=False, debug=False,
                   enable_asserts=True, num_devices=N_CORES)

    xt_ap = nc.dram_tensor("xt", [P, N], gemm_dt, kind="ExternalInput").ap()
    wneg2_ap = nc.dram_tensor("wneg2", [P, U], gemm_dt, kind="ExternalInput").ap()
    x2_ap = nc.dram_tensor("x2", [P, N_TILES], mybir.dt.float32,
                           kind="ExternalInput").ap()
    w2_ap = nc.dram_tensor("w2", [1, U], mybir.dt.float32,
                           kind="ExternalInput").ap()
    out_ap = nc.dram_tensor("out", [N, U], mybir.dt.float32,
                            kind="ExternalOutput").ap()

    with tile.TileContext(nc) as tc:
        with (
            tc.tile_pool(name="singles", bufs=1) as singles,
            tc.tile_pool(name="xchunks", bufs=N // XT_CHUNK) as xchunks,
            tc.tile_pool(name="psum", bufs=4, space="PSUM") as psum_pool,
            tc.tile_pool(name="outs", bufs=4) as out_pool,
        ):
            # Load order matters: the first n-tile's matmul needs wneg2 +
            # xt chunk 0, so issue those first; the rest overlaps compute.
            wneg2_sb = singles.tile([P, U], gemm_dt, tag="wneg2")
            nc.sync.dma_start(wneg2_sb[:], wneg2_ap[:])
            xt_sbs = []
            for ci in range(N // XT_CHUNK):
                t = xchunks.tile([P, XT_CHUNK], gemm_dt, tag=f"xt{ci}")
                xt_sbs.append(t)
            nc.sync.dma_start(xt_sbs[0][:], xt_ap[:, 0:XT_CHUNK])
            x2_sb = singles.tile([P, N_TILES], mybir.dt.float32, tag="x2")
            nc.sync.dma_start(x2_sb[:], x2_ap[:])
            w2row_sb = singles.tile([1, U], mybir.dt.float32, tag="w2row")
            nc.sync.dma_start(w2row_sb[:], w2_ap[:])
            for ci in range(1, N // XT_CHUNK):
                nc.sync.dma_start(xt_sbs[ci][:],
                                  xt_ap[:, ci * XT_CHUNK:(ci + 1) * XT_CHUNK])

            # |w_u|^2 broadcast to all partitions on the otherwise-idle
            # GpSimd engine (4 KiB DMA instead of a 512 KiB full plane).
            w2p_sb = singles.tile([P, U], mybir.dt.float32, tag="w2p")
            nc.gpsimd.partition_broadcast(w2p_sb[:], w2row_sb[:])

            tiles_per_chunk = XT_CHUNK // P
            for j in range(N_TILES):
                chunk = xt_sbs[j // tiles_per_chunk]
                col0 = (j % tiles_per_chunk) * P
                lhsT = chunk[:, col0:col0 + P]

                acc = psum_pool.tile([P, U], mybir.dt.float32, tag="acc")
                for h in range(U // U_HALF):
                    nc.tensor.matmul(
                        acc[:, h * U_HALF:(h + 1) * U_HALF],
                        lhsT,
                        wneg2_sb[:, h * U_HALF:(h + 1) * U_HALF],
                        start=True, stop=True,
                    )

                o = out_pool.tile([P, U], mybir.dt.float32, tag="o")
                # o = (acc * 1 + x2[:, j])  (per-partition bias) on ScalarE
                nc.scalar.activation(
                    out=o[:], in_=acc[:],
                    func=mybir.ActivationFunctionType.Identity,
                    bias=x2_sb[:, j:j + 1], scale=1.0,
                )
                # o += w2p on VectorE
                nc.vector.tensor_add(o[:], o[:], w2p_sb[:])
                nc.sync.dma_start(out_ap[j * P:(j + 1) * P, :], o[:])

    nc.compile()
    return nc


_CACHED_NC = None


def _get_nc():
    global _CACHED_NC
    if _CACHED_NC is None:
        _CACHED_NC = build_bass()
    return _CACHED_NC


def make_in_maps(x, w, gemm_np=None):
    """Host-side shard + precompute: per-core input dict list."""
    gemm_np = gemm_np or GEMM_NP
    x = np.asarray(x, dtype=np.float32)
    w = np.asarray(w, dtype=np.float32)
    wneg2 = (-2.0 * w).astype(gemm_np)
    w2 = (w.astype(np.float64) ** 2).sum(axis=0).astype(np.float32)
    w2row = w2.reshape(1, U)
    in_maps = []
    for c in range(N_CORES):
        xs = x[c]                                    # [4096, 128]
        xt = np.ascontiguousarray(xs.T).astype(gemm_np)       # [128, 4096]
        x2 = (xs ** 2).sum(axis=1, dtype=np.float32)          # [4096]
        x2cols = np.ascontiguousarray(x2.reshape(N_TILES, P).T)  # [128, 32]
        in_maps.append({"xt": xt, "wneg2": wneg2, "x2": x2cols, "w2": w2row})
    return in_maps


def run(x, w, trace=False):
    _install_ntff_hook()
    nc = _get_nc()
    in_maps = make_in_maps(x, w)
    res = run_bass_kernel_spmd(nc, in_maps, core_ids=list(range(N_CORES)),
                               trace=trace)
    out = np.stack([res.results[c]["out"] for c in range(N_CORES)], axis=0)
    return out, res


def kernel(x, w):
    out, _ = run(x, w, trace=False)
    return out
